# revision 3
# baseline (speedup 1.0000x reference)
"""Differentiable H.264 (8x8 DCT quantize roundtrip on luminance) Trainium2 kernel.

Self-contained: builds a Bass/Tile kernel, shards batch 8 across 8 NeuronCores
(pure data parallel), runs via run_bass_kernel_spmd, returns full output.

Algorithm per core (one image, 3x1080x1920 f32):
  y   = 0.114 b + 0.587 g + 0.299 r          (fused into first DCT stage)
  C   = Bh @ Y @ Bw^T   per 8x8 block        (2D DCT, orthonormal)
  Cq  = round(C / (q+1e-8)) * q
  yd  = IDCT2(Cq - C)                        (= y_rec - y, by linearity)
  out_c = clip(x_c + w_c * yd, 0, 255)

Implementation: row strips of 128 rows (last strip 56 valid rows padded to 64).
All four 8-point DCT applications are 128x128 block-diagonal matmuls on the
partition axis; the H<->W axis swap uses the DVE 32x32 blockwise stream
transpose, whose mixed layout is consistent for a block-diagonal transform
(quant pattern stays q[f%8, p%8]). The per-channel scale w_c is folded into
the second-IDCT stationaries, and `+ x_c` is a second matmul (identity
stationary) accumulating into the same PSUM group, so the only remaining
elementwise work is quantization (3 TT + 1 round) and the final clip.
"""

import numpy as np

H, W = 1080, 1920
B, CH = 8, 3
N_CORES = 8
CHUNK = 480  # matmul free-dim chunk (4 per 1920, fp32 <= 512, 1 PSUM bank)
MAGIC = 12582912.0  # 1.5*2^23: (x+M)-M == round-half-even for |x| < 2^22
CW = [0.114, 0.587, 0.299]  # BGR -> Y weights (channel order 0,1,2)

_BASE_QUANT = np.array([
    [16, 11, 10, 16, 24, 40, 51, 61],
    [12, 12, 14, 19, 26, 58, 60, 55],
    [14, 13, 16, 24, 40, 57, 69, 56],
    [14, 17, 22, 29, 51, 87, 80, 62],
    [18, 22, 37, 56, 68, 109, 103, 77],
    [24, 35, 55, 64, 81, 104, 113, 92],
    [49, 64, 78, 87, 103, 121, 120, 101],
    [72, 92, 95, 98, 112, 100, 103, 99]], dtype=np.float32)
QF = 28


def _consts():
    scale = 50.0 / max(1, QF) if QF < 25 else 200.0 - 2 * QF
    q = np.maximum(_BASE_QUANT * scale / 50.0, 1.0).astype(np.float32)
    n = np.arange(8, dtype=np.float32)
    bas = (np.sqrt(np.float32(2.0 / 8)) *
           np.cos(np.float32(np.pi) * n[:, None] * (2 * n[None, :] + 1) / 16.0)
           ).astype(np.float32)
    bas[0, :] = np.sqrt(np.float32(1.0 / 8))
    qe = (q + 1e-8).astype(np.float32)

    def blkdiag(b):
        out = np.zeros((128, 128), np.float32)
        for i in range(16):
            out[8*i:8*i+8, 8*i:8*i+8] = b
        return out

    sf = blkdiag(bas.T)  # lhsT for forward stages: out = (I (x) basis) @ rhs
    si = blkdiag(bas)    # lhsT for inverse stages
    # wf: [128, 512] = w_b*sf | w_g*sf | w_r*sf | sf     (A1 x3, A2)
    wf = np.concatenate([np.float32(c) * sf for c in CW] + [sf], axis=1)
    # wi: [128, 640] = si | w_b*si | w_g*si | w_r*si | I (D1, D2 x3, add)
    wi = np.concatenate([si] + [np.float32(c) * si for c in CW] +
                        [np.eye(128, dtype=np.float32)], axis=1)
    # rq: [128, 16] = R8 | Q8 with R8[p,j] = 1/qe[j, p%8], Q8[p,j] = q[j, p%8]
    p = np.arange(128) % 8
    r8 = (np.float32(1.0) / qe[:, p]).T.astype(np.float32)   # [128, 8]
    q8 = q[:, p].T.astype(np.float32)
    rq = np.concatenate([r8, q8], axis=1)
    return wf.astype(np.float32), wi.astype(np.float32), rq.astype(np.float32)


def build_nc(reps=1):
    import concourse.bacc as bacc
    import concourse.tile as tile
    import concourse.bass as bass
    from concourse import mybir
    from concourse.alu_op_type import AluOpType as alu

    f32 = mybir.dt.float32
    nc = bacc.Bacc("TRN2", target_bir_lowering=False, debug=False,
                   num_devices=N_CORES)
    x = nc.dram_tensor("x", [CH, H, W], f32, kind="ExternalInput")
    wf = nc.dram_tensor("wf", [128, 512], f32, kind="ExternalInput")
    wi = nc.dram_tensor("wi", [128, 640], f32, kind="ExternalInput")
    rq = nc.dram_tensor("rq", [128, 16], f32, kind="ExternalInput")
    y = nc.dram_tensor("y", [CH, H, W], f32, kind="ExternalOutput")

    strips = [(k * 128, 128, 128) for k in range(8)] + [(1024, 64, 56)]
    nch = W // CHUNK

    with tile.TileContext(nc) as tc:
        with (
            tc.tile_pool(name="consts", bufs=1) as cpool,
            tc.tile_pool(name="xin", bufs=2) as xpool,
            tc.tile_pool(name="trans", bufs=3) as tpool,
            tc.tile_pool(name="quant", bufs=3) as qpool,
            tc.tile_pool(name="csb", bufs=2) as cspool,
            tc.tile_pool(name="outs", bufs=2) as opool,
            tc.tile_pool(name="ps", bufs=8, space="PSUM") as pspool,
        ):
            cw = cpool.tile([128, 512], f32)
            nc.sync.dma_start(out=cw, in_=wf[:, :])
            ci = cpool.tile([128, 640], f32)
            nc.sync.dma_start(out=ci, in_=wi[:, :])
            crq = cpool.tile([128, 16], f32)
            nc.sync.dma_start(out=crq, in_=rq[:, :])

            def bcast_rq(off8, P):
                # [P, W//8, 8] AP over crq with step-0 repeat along W//8
                base = crq[:P, off8:off8 + 8]
                return bass.AP(tensor=base.tensor, offset=base.offset,
                               ap=[list(base.ap[0]), [0, W // 8],
                                   list(base.ap[1])])

            s3 = lambda ap: ap.rearrange("p (a b) -> p a b", b=8)

            for _ in range(reps):
                for (r0, P, valid) in strips:
                    xt = []
                    for c in range(CH):
                        t = xpool.tile([P, W], f32, tag=f"x{c}")
                        if valid < P:
                            # pad rows must be finite: 32-aligned partition
                            # bases only, so zero the whole tile first
                            nc.vector.memset(t[:, :], 0.0)
                        nc.sync.dma_start(out=t[:valid, :],
                                          in_=x[c, r0:r0 + valid, :])
                        xt.append(t)

                    # A1: U = (I x basis) @ (w.x)  [lum fused], per chunk
                    us = tpool.tile([P, W], f32, tag="t")
                    for j in range(nch):
                        sl = slice(j * CHUNK, (j + 1) * CHUNK)
                        u = pspool.tile([P, CHUNK], f32, tag="ps")
                        for c in range(CH):
                            nc.tensor.matmul(u, cw[:P, c*128:c*128 + P],
                                             xt[c][:, sl],
                                             start=(c == 0), stop=(c == 2))
                        nc.scalar.copy(us[:, sl], u)

                    # blockwise transpose (h <-> w within 32x32 blocks)
                    tt = tpool.tile([P, W], f32, tag="t")
                    nc.vector.transpose(tt, us)

                    # A2: C = (I x basis) @ T, then PSUM->SBUF copy
                    cs = cspool.tile([P, W], f32, tag="cs")
                    for j in range(nch):
                        sl = slice(j * CHUNK, (j + 1) * CHUNK)
                        cps = pspool.tile([P, CHUNK], f32, tag="ps")
                        nc.tensor.matmul(cps, cw[:P, 384:384 + P], tt[:, sl],
                                         start=True, stop=True)
                        nc.scalar.copy(cs[:, sl], cps)

                    # quantization: qerr = round(C/(q+1e-8))*q - C
                    sq = qpool.tile([P, W], f32, tag="q")
                    nc.vector.tensor_tensor(s3(sq), s3(cs), bcast_rq(0, P),
                                            alu.mult)
                    rr = qpool.tile([P, W], f32, tag="q")
                    nc.vector.tensor_scalar(rr, sq, MAGIC, MAGIC,
                                            alu.add, alu.subtract)
                    ee = qpool.tile([P, W], f32, tag="q")
                    nc.gpsimd.tensor_tensor(s3(ee), s3(rr), bcast_rq(8, P),
                                            alu.mult)
                    qt = qpool.tile([P, W], f32, tag="q")
                    nc.gpsimd.tensor_tensor(qt, ee, cs, alu.subtract)

                    # D1: IDCT along w (still in transposed layout)
                    d1s = tpool.tile([P, W], f32, tag="t")
                    for j in range(nch):
                        sl = slice(j * CHUNK, (j + 1) * CHUNK)
                        d1 = pspool.tile([P, CHUNK], f32, tag="ps")
                        nc.tensor.matmul(d1, ci[:P, :P], qt[:, sl],
                                         start=True, stop=True)
                        nc.scalar.copy(d1s[:, sl], d1)

                    # blockwise transpose back
                    et = tpool.tile([P, W], f32, tag="t")
                    nc.vector.transpose(et, d1s)

                    # D2 + add: out_c = w_c * (I x basis^T) @ E + x_c  (PSUM)
                    # then clip on DVE straight from PSUM into SBUF out tile
                    for c in range(CH):
                        ot = opool.tile([P, W], f32, tag=f"o{c}")
                        for j in range(nch):
                            sl = slice(j * CHUNK, (j + 1) * CHUNK)
                            ops = pspool.tile([P, CHUNK], f32, tag="ps")
                            nc.tensor.matmul(
                                ops, ci[:P, (1 + c)*128:(1 + c)*128 + P],
                                et[:, sl], start=True, stop=False)
                            nc.tensor.matmul(
                                ops, ci[:P, 512:512 + P], xt[c][:, sl],
                                start=False, stop=True)
                            nc.vector.tensor_scalar(ot[:, sl], ops,
                                                    0.0, 255.0,
                                                    alu.max, alu.min)
                        nc.sync.dma_start(out=y[c, r0:r0 + valid, :],
                                          in_=ot[:valid, :])

    nc.compile()
    return nc


_NC_CACHE = {}


def _get_nc(reps=1):
    if reps not in _NC_CACHE:
        _NC_CACHE[reps] = build_nc(reps)
    return _NC_CACHE[reps]


def kernel(x):
    """x: (8, 3, 1080, 1920) float32 -> (8, 3, 1080, 1920) float32."""
    from concourse.bass_utils import run_bass_kernel_spmd

    x = np.asarray(x, dtype=np.float32)
    assert x.shape == (B, CH, H, W)
    wf, wi, rq = _consts()
    nc = _get_nc(1)
    in_maps = [{"x": x[b], "wf": wf, "wi": wi, "rq": rq} for b in range(B)]
    res = run_bass_kernel_spmd(nc, in_maps, list(range(N_CORES)))
    out = np.stack([res.results[b]["y"] for b in range(B)], axis=0)
    return out
